# revision 23
# baseline (speedup 1.0000x reference)
"""Trainium2 Bass kernel: batched inverse of homogeneous affine transforms.

Problem: trf (B, 3, 4) fp32 "shift" affines. Padded M = [[I3 + dA, t], [0, 1]].
Output = top 3 rows of M^-1 = [A^-1 | -A^-1 t] where A = I3 + dA.

Closed form via the column-cross-product adjugate:
    Z[3r+j]  = P[3r+j] - Q[3r+j]   (cross(a_{r+1}, a_{r+2}) components)
    det      = a1 . Z[0:3] ; O = Z * (1/det) ; w_r = sum_j O[r][j] * (-t_j)

Layout: PLANAR per partition. Host pre-permutes each core's (BL, 12) slab to
(nch, 128, 12, C): partition p holds 12 contiguous planes of C consecutive
matrices. Every engine op then runs on dense step-1 inner runs (measured ~2x
faster on DVE than the stride-12 interleaved layout) while each DMA still
moves one contiguous 24KB run per partition.

Plane permutation POS (slot -> matrix position) was chosen by combinatorial
search so the 9 Q-products batch as 3 ops, P row 0 batches, and tm batches
(arithmetic-progression plane strides).

Engine split: ALL 2-input tensor ops run on the DVE. GPSIMD shares its SBUF
port with the DVE (the POOL slot) — measured combined throughput with both
engines active equals DVE-alone, so offloading to GPSIMD is a net loss. ACT
(its own port) runs the 1-input work (diag+1, t negation) in parallel. The
det->recip->scale chain stays entirely on V (a cross-engine replication copy
costs ~1.2-1.8us in semaphore latency per chunk); emission is software-
pipelined so V runs chunk n+1's products while ACT works and DMAs stream.
"""

import numpy as np

B = 4_194_304
NCORES = 8
BL = B // NCORES  # 524288 matrices per core
P = 128
C = 512           # matrices per partition per chunk
NCH = BL // (P * C)  # 8 chunks

# slot -> input position (position = 4*r + col, row-major (3,4))
POS = [5, 0, 4, 8, 9, 10, 2, 6, 1, 3, 7, 11]
# output plane k -> output position: planes 0..8 = O[r][j] at 4r+j, 9..11 = w_r
OPOS = [0, 1, 2, 4, 5, 6, 8, 9, 10, 3, 7, 11]

# P products (out plane 3r+j in po block), as (out, in0_slot, in1_slot):
# row 0 batched: out {0,1,2}, in0 [0,4,8] (step 4), in1 [5,6,7] (step 1)
# rows 1,2 as pairs (2-term progressions are always affine):
# (out_base, out_step, in0_base, in0_step, in1_base, in1_step)
P_PAIRS = [
    (3, 1, 7, -2, 3, -2),   # (3,7,3), (4,5,1)
    (5, 1, 6, -4, 2, 2),    # (5,6,2), (6,2,4)
    (7, 1, 3, -2, 8, -8),   # (7,3,8), (8,1,0)
]
# Q products batched by j: (out_base, out_step, in0_base, in0_step, in1_base, in1_step)
Q_BATCH = [
    (0, 3, 7, -2, 4, -2),   # j=0: out {0,3,6}, in0 [7,5,3], in1 [4,2,0]
    (1, 3, 5, -2, 8, -2),   # j=1: out {1,4,7}, in0 [5,3,1], in1 [8,6,4]
    (2, 3, 0, 1, 6, 1),     # j=2: out {2,5,8}, in0 [0,1,2], in1 [6,7,8]
]

# engine plan: op -> "v" (DVE) / "g" (GPSIMD). GPSIMD's SBUF port is shared
# with the DVE (POOL slot): measured combined V+G throughput during overlap
# equals V-alone, so all 2-input work stays on V; ACT (own port) runs 1-input.
DEFAULT_PLAN = {
    "p0": "v",                               # batched P row 0
    **{f"pp{i}": "v" for i in range(3)},     # P pairs (rows 1,2)
    **{f"q{j}": "v" for j in range(3)},      # batched Q
    "z": "v",
    "tm": "v",
    "det1": "v",
    "det2": "v",
    "s": "v",
    "w": "v",
    "scale": "v",
    **{f"wp{r}": "v" for r in range(3)},
}


def _V(base_ap, off, dims):
    """Strided view of a tile: dims = [(step, count), ...] free dims, last
    dim innermost. Offset in elements."""
    import concourse.bass as bass

    return bass.AP(
        base_ap.tensor,
        base_ap.offset + off,
        [list(base_ap.ap[0])] + [[int(s), int(n)] for s, n in dims],
    )


def build_nc(bl=BL, c=C, plan=None):
    import concourse.bass as bass
    import concourse.bacc as bacc
    import concourse.mybir as mybir
    from concourse.tile import TileContext

    plan = dict(DEFAULT_PLAN, **(plan or {}))
    f32 = mybir.dt.float32
    nch = bl // (P * c)
    assert bl == nch * P * c

    nc = bacc.Bacc()
    # DRAM layout (host-permuted): (nch*128, 12*C) — row = (chunk, partition),
    # 12 planar planes of C floats contiguous per row.
    trf = nc.declare_dram_parameter("trf", [nch * P, 12 * c], f32, isOutput=False)
    out = nc.declare_dram_parameter("out", [nch * P, 12 * c], f32, isOutput=True)
    trf_t = trf.ap().rearrange("(n p) f -> n p f", p=P)
    out_t = out.ap().rearrange("(n p) f -> n p f", p=P)

    eng = None
    state = {}

    def prefetch(n, tio):
        # DMA-in + diag (ACT) — issued one stage early so the load and the
        # diag pass are done before the products need them
        t = tio.tile([P, 12 * c], f32, tag="t")
        # split load: products only need planes 0-8, so they can start
        # before the t-planes (9-11) land
        nc.sync.dma_start(
            out=_V(t, 0, [(1, 9 * c)]), in_=_V(trf_t[n], 0, [(1, 9 * c)])
        )
        nc.sync.dma_start(
            out=_V(t, 9 * c, [(1, 3 * c)]), in_=_V(trf_t[n], 9 * c, [(1, 3 * c)])
        )
        d01 = _V(t, 0, [(1, 2 * c)])
        nc.scalar.add(d01, d01, 1.0)
        d5 = _V(t, 5 * c, [(1, c)])
        nc.scalar.add(d5, d5, 1.0)
        state[n] = {"t": t}

    def head(n, io, tmp):
        # all 18 products (V)
        st = state[n]
        t = st["t"]
        po = io.tile([P, 12 * c], f32, tag="po")  # P/Z/O planes 0..8, w 9..11
        qq = tmp.tile([P, 9 * c], f32, tag="qq")  # Q then wp
        st["po"], st["qq"] = po, qq
        eng[plan["p0"]].tensor_mul(
            _V(po, 0, [(c, 3), (1, c)]),
            _V(t, 0, [(4 * c, 3), (1, c)]),
            _V(t, 5 * c, [(c, 3), (1, c)]),
        )
        for i, (ob, os_, a0, s0, b0, s1) in enumerate(P_PAIRS):
            eng[plan[f"pp{i}"]].tensor_mul(
                _V(po, ob * c, [(os_ * c, 2), (1, c)]),
                _V(t, a0 * c, [(s0 * c, 2), (1, c)]),
                _V(t, b0 * c, [(s1 * c, 2), (1, c)]),
            )
        for j, (ob, os_, a0, s0, b0, s1) in enumerate(Q_BATCH):
            eng[plan[f"q{j}"]].tensor_mul(
                _V(qq, ob * c, [(os_ * c, 3), (1, c)]),
                _V(t, a0 * c, [(s0 * c, 3), (1, c)]),
                _V(t, b0 * c, [(s1 * c, 3), (1, c)]),
            )

    def mid(n, tmp):
        # Z, det chain, recip (V); rdet9 fan-out + tneg (ACT)
        st = state[n]
        t, po, qq = st["t"], st["po"], st["qq"]
        pf = _V(po, 0, [(1, 9 * c)])
        eng[plan["z"]].tensor_sub(pf, pf, _V(qq, 0, [(1, 9 * c)]))

        tm = tmp.tile([P, 3 * c], f32, tag="tm")
        st["tm"] = tm
        eng[plan["tm"]].tensor_mul(
            _V(tm, 0, [(c, 3), (1, c)]),
            _V(t, c, [(c, 3), (1, c)]),
            _V(po, 0, [(c, 3), (1, c)]),
        )
        det = tmp.tile([P, c], f32, tag="det")
        eng[plan["det1"]].tensor_add(
            det[:], _V(tm, 0, [(1, c)]), _V(tm, c, [(1, c)])
        )
        eng[plan["det2"]].tensor_add(det[:], det[:], _V(tm, 2 * c, [(1, c)]))

        # rdet = 1/det (single custom DVE op, ~4e-6 rel err; det~1 so no
        # edge cases). No replication: the scale stage reads plane 0 directly
        # as nine single-plane ops, keeping the whole det->scale chain on V
        # (a cross-engine copy costs more in semaphore latency than the
        # extra op overheads).
        rdet = tmp.tile([P, c], f32, tag="rdet")
        st["rdet"] = rdet
        nc.vector.reciprocal_approx_fast(rdet[:], det[:])

        # tneg: t planes 9..11 *= -1 (in place, ACT)
        tp = _V(t, 9 * c, [(1, 3 * c)])
        nc.scalar.mul(tp, tp, -1.0)

    def tail(n):
        # scale, wp, w sums, output DMAs — interleaved per output row so each
        # row's O-DMA ships as soon as its scale+wp are done, shrinking the
        # end-of-kernel tail
        st = state.pop(n)
        t, po, qq, tm, rdet = st["t"], st["po"], st["qq"], st["tm"], st["rdet"]
        for r in range(3):
            for k in range(3 * r, 3 * r + 3):
                eng[plan["scale"]].tensor_mul(
                    _V(po, k * c, [(1, c)]),
                    _V(po, k * c, [(1, c)]),
                    rdet[:],
                )
            eng[plan[f"wp{r}"]].tensor_mul(
                _V(qq, 3 * r * c, [(1, 3 * c)]),
                _V(po, 3 * r * c, [(1, 3 * c)]),
                _V(t, 9 * c, [(1, 3 * c)]),
            )
            nc.sync.dma_start(
                out=_V(out_t[n], 3 * r * c, [(1, 3 * c)]),
                in_=_V(po, 3 * r * c, [(1, 3 * c)]),
            )
        # w_r = wp[3r] + wp[3r+1] + wp[3r+2] -> po planes 9..11
        # (s scratch reuses tm, dead after the det sums)
        eng[plan["s"]].tensor_add(
            _V(tm, 0, [(c, 3), (1, c)]),
            _V(qq, 0, [(3 * c, 3), (1, c)]),
            _V(qq, c, [(3 * c, 3), (1, c)]),
        )
        eng[plan["w"]].tensor_add(
            _V(po, 9 * c, [(c, 3), (1, c)]),
            _V(tm, 0, [(c, 3), (1, c)]),
            _V(qq, 2 * c, [(3 * c, 3), (1, c)]),
        )
        nc.sync.dma_start(
            out=_V(out_t[n], 9 * c, [(1, 3 * c)]),
            in_=_V(po, 9 * c, [(1, 3 * c)]),
        )

    with TileContext(nc) as tc:
        with (
            tc.tile_pool(name="tio", bufs=3) as tio,
            tc.tile_pool(name="io", bufs=2) as io,
            tc.tile_pool(name="tmp", bufs=2) as tmp,
        ):
            eng = {"v": nc.vector, "g": nc.gpsimd}
            # software pipeline: V runs chunk n+1's products while ACT does
            # chunk n's rdet9 fan-out, so V never stalls on the copy chain.
            # prefetch(n+1) is emitted after tail(n-1) so every prior-chunk
            # consumer of its recycled buffers is already known to Tile.
            prefetch(0, tio)
            for n in range(nch):
                head(n, io, tmp)
                if n >= 1:
                    tail(n - 1)
                mid(n, tmp)
                if n + 1 < nch:
                    prefetch(n + 1, tio)
            tail(nch - 1)

    return nc


_CACHE = {}


def _get_nc():
    if "nc" not in _CACHE:
        nc = build_nc()
        nc.finalize()
        _CACHE["nc"] = nc
    return _CACHE["nc"]


def _shard_inputs(trf):
    """(B,3,4) -> per-core (nch*128, 12*C) planar slabs."""
    x = np.ascontiguousarray(np.asarray(trf, dtype=np.float32)).reshape(
        NCORES, NCH, P, C, 12
    )
    # permute matrix positions into plane slots, planes outer, matrices inner
    xp = x[:, :, :, :, POS].transpose(0, 1, 2, 4, 3)  # (8, nch, 128, 12, C)
    xp = np.ascontiguousarray(xp).reshape(NCORES, NCH * P, 12 * C)
    return xp


def _unshard_output(outs):
    """per-core (nch*128, 12*C) planar -> (B, 3, 4)."""
    o = outs.reshape(NCORES, NCH, P, 12, C).transpose(0, 1, 2, 4, 3)
    full = np.empty((NCORES, NCH, P, C, 12), dtype=np.float32)
    full[..., OPOS] = o
    return full.reshape(B, 3, 4)


def run(trf, trace=False, **spmd_kwargs):
    """Shard, run on 8 cores, gather. Returns (output, BassKernelResults)."""
    from concourse.bass_utils import run_bass_kernel_spmd

    xp = _shard_inputs(trf)
    in_maps = [{"trf": xp[i]} for i in range(NCORES)]
    nc = _get_nc()
    res = run_bass_kernel_spmd(
        nc, in_maps, list(range(NCORES)), trace=trace, **spmd_kwargs
    )
    outs = np.stack([np.asarray(res.results[i]["out"]) for i in range(NCORES)])
    return _unshard_output(outs).astype(np.float32), res


def kernel(trf):
    return run(trf)[0]


# revision 24
# speedup vs baseline: 1.1776x; 1.1776x over previous
"""Trainium2 Bass kernel: batched inverse of homogeneous affine transforms.

Problem: trf (B, 3, 4) fp32 "shift" affines. Padded M = [[I3 + dA, t], [0, 1]].
Output = top 3 rows of M^-1 = [A^-1 | -A^-1 t] where A = I3 + dA.

Closed form via the column-cross-product adjugate:
    Z[3r+j]  = P[3r+j] - Q[3r+j]   (cross(a_{r+1}, a_{r+2}) components)
    det      = a1 . Z[0:3] ; O = Z * (1/det) ; w_r = sum_j O[r][j] * (-t_j)

Layout: PLANAR per partition. Host pre-permutes each core's (BL, 12) slab to
(nch, 128, 12, C): partition p holds 12 contiguous planes of C consecutive
matrices. Every engine op then runs on dense step-1 inner runs (measured ~2x
faster on DVE than the stride-12 interleaved layout) while each DMA still
moves one contiguous 24KB run per partition.

Plane permutation POS (slot -> matrix position) was chosen by combinatorial
search so the 9 Q-products batch as 3 ops, P row 0 batches, and tm batches
(arithmetic-progression plane strides).

Engine split: ALL 2-input tensor ops run on the DVE. GPSIMD shares its SBUF
port with the DVE (the POOL slot) — measured combined throughput with both
engines active equals DVE-alone, so offloading to GPSIMD is a net loss. ACT
(its own port) runs the 1-input work (diag+1, t negation) in parallel. The
det->recip->scale chain stays entirely on V (a cross-engine replication copy
costs ~1.2-1.8us in semaphore latency per chunk); emission is software-
pipelined so V runs chunk n+1's products while ACT works and DMAs stream.
"""

import numpy as np

B = 4_194_304
NCORES = 8
BL = B // NCORES  # 524288 matrices per core
P = 128
C = 512           # matrices per partition per chunk
NCH = BL // (P * C)  # 8 chunks

# slot -> input position (position = 4*r + col, row-major (3,4))
POS = [5, 0, 4, 8, 9, 10, 2, 6, 1, 3, 7, 11]
# output plane k -> output position: planes 0..8 = O[r][j] at 4r+j, 9..11 = w_r
OPOS = [0, 1, 2, 4, 5, 6, 8, 9, 10, 3, 7, 11]

# P products (out plane 3r+j in po block), as (out, in0_slot, in1_slot):
# row 0 batched: out {0,1,2}, in0 [0,4,8] (step 4), in1 [5,6,7] (step 1)
# rows 1,2 as pairs (2-term progressions are always affine):
# (out_base, out_step, in0_base, in0_step, in1_base, in1_step)
P_PAIRS = [
    (3, 1, 7, -2, 3, -2),   # (3,7,3), (4,5,1)
    (5, 1, 6, -4, 2, 2),    # (5,6,2), (6,2,4)
    (7, 1, 3, -2, 8, -8),   # (7,3,8), (8,1,0)
]
# Q products batched by j: (out_base, out_step, in0_base, in0_step, in1_base, in1_step)
Q_BATCH = [
    (0, 3, 7, -2, 4, -2),   # j=0: out {0,3,6}, in0 [7,5,3], in1 [4,2,0]
    (1, 3, 5, -2, 8, -2),   # j=1: out {1,4,7}, in0 [5,3,1], in1 [8,6,4]
    (2, 3, 0, 1, 6, 1),     # j=2: out {2,5,8}, in0 [0,1,2], in1 [6,7,8]
]

# engine plan: op -> "v" (DVE) / "g" (GPSIMD). GPSIMD's SBUF port is shared
# with the DVE (POOL slot): measured combined V+G throughput during overlap
# equals V-alone, so all 2-input work stays on V; ACT (own port) runs 1-input.
DEFAULT_PLAN = {
    "p0": "v",                               # batched P row 0
    **{f"pp{i}": "v" for i in range(3)},     # P pairs (rows 1,2)
    **{f"q{j}": "v" for j in range(3)},      # batched Q
    "z": "v",
    "tm": "v",
    "det1": "v",
    "det2": "v",
    "s": "v",
    "w": "v",
    "scale": "v",
    **{f"wp{r}": "v" for r in range(3)},
}


def _V(base_ap, off, dims):
    """Strided view of a tile: dims = [(step, count), ...] free dims, last
    dim innermost. Offset in elements."""
    import concourse.bass as bass

    return bass.AP(
        base_ap.tensor,
        base_ap.offset + off,
        [list(base_ap.ap[0])] + [[int(s), int(n)] for s, n in dims],
    )


def build_nc(bl=BL, c=C, plan=None):
    import concourse.bass as bass
    import concourse.bacc as bacc
    import concourse.mybir as mybir
    from concourse.tile import TileContext

    plan = dict(DEFAULT_PLAN, **(plan or {}))
    f32 = mybir.dt.float32
    nch = bl // (P * c)
    assert bl == nch * P * c

    nc = bacc.Bacc()
    # DRAM layout (host-permuted): (nch*128, 12*C) — row = (chunk, partition),
    # 12 planar planes of C floats contiguous per row.
    trf = nc.declare_dram_parameter("trf", [nch * P, 12 * c], f32, isOutput=False)
    out = nc.declare_dram_parameter("out", [nch * P, 12 * c], f32, isOutput=True)
    trf_t = trf.ap().rearrange("(n p) f -> n p f", p=P)
    out_t = out.ap().rearrange("(n p) f -> n p f", p=P)

    eng = None
    state = {}

    def prefetch(n, tio):
        # DMA-in + diag (ACT) — issued one stage early so the load and the
        # diag pass are done before the products need them
        t = tio.tile([P, 12 * c], f32, tag="t")
        # split load: products only need planes 0-8, so they can start
        # before the t-planes (9-11) land
        nc.sync.dma_start(
            out=_V(t, 0, [(1, 9 * c)]), in_=_V(trf_t[n], 0, [(1, 9 * c)])
        )
        nc.sync.dma_start(
            out=_V(t, 9 * c, [(1, 3 * c)]), in_=_V(trf_t[n], 9 * c, [(1, 3 * c)])
        )
        d01 = _V(t, 0, [(1, 2 * c)])
        nc.scalar.add(d01, d01, 1.0)
        d5 = _V(t, 5 * c, [(1, c)])
        nc.scalar.add(d5, d5, 1.0)
        state[n] = {"t": t}

    def head(n, io, tmp):
        # all 18 products (V)
        st = state[n]
        t = st["t"]
        po = io.tile([P, 12 * c], f32, tag="po")  # P/Z/O planes 0..8, w 9..11
        qq = tmp.tile([P, 9 * c], f32, tag="qq")  # Q then wp
        st["po"], st["qq"] = po, qq
        eng[plan["p0"]].tensor_mul(
            _V(po, 0, [(c, 3), (1, c)]),
            _V(t, 0, [(4 * c, 3), (1, c)]),
            _V(t, 5 * c, [(c, 3), (1, c)]),
        )
        for i, (ob, os_, a0, s0, b0, s1) in enumerate(P_PAIRS):
            eng[plan[f"pp{i}"]].tensor_mul(
                _V(po, ob * c, [(os_ * c, 2), (1, c)]),
                _V(t, a0 * c, [(s0 * c, 2), (1, c)]),
                _V(t, b0 * c, [(s1 * c, 2), (1, c)]),
            )
        for j, (ob, os_, a0, s0, b0, s1) in enumerate(Q_BATCH):
            eng[plan[f"q{j}"]].tensor_mul(
                _V(qq, ob * c, [(os_ * c, 3), (1, c)]),
                _V(t, a0 * c, [(s0 * c, 3), (1, c)]),
                _V(t, b0 * c, [(s1 * c, 3), (1, c)]),
            )

    def mid(n, tmp):
        # Z, det chain, recip (V); rdet9 fan-out + tneg (ACT)
        st = state[n]
        t, po, qq = st["t"], st["po"], st["qq"]
        pf = _V(po, 0, [(1, 9 * c)])
        eng[plan["z"]].tensor_sub(pf, pf, _V(qq, 0, [(1, 9 * c)]))

        tm = tmp.tile([P, 3 * c], f32, tag="tm")
        st["tm"] = tm
        eng[plan["tm"]].tensor_mul(
            _V(tm, 0, [(c, 3), (1, c)]),
            _V(t, c, [(c, 3), (1, c)]),
            _V(po, 0, [(c, 3), (1, c)]),
        )
        det = tmp.tile([P, c], f32, tag="det")
        eng[plan["det1"]].tensor_add(
            det[:], _V(tm, 0, [(1, c)]), _V(tm, c, [(1, c)])
        )
        eng[plan["det2"]].tensor_add(det[:], det[:], _V(tm, 2 * c, [(1, c)]))

        # rdet = 1/det (single custom DVE op, ~4e-6 rel err; det~1 so no
        # edge cases). No replication: the scale stage reads plane 0 directly
        # as nine single-plane ops, keeping the whole det->scale chain on V
        # (a cross-engine copy costs more in semaphore latency than the
        # extra op overheads).
        rdet = tmp.tile([P, c], f32, tag="rdet")
        st["rdet"] = rdet
        nc.vector.reciprocal_approx_fast(rdet[:], det[:])

        # tneg: t planes 9..11 *= -1 (in place, ACT)
        tp = _V(t, 9 * c, [(1, 3 * c)])
        nc.scalar.mul(tp, tp, -1.0)

    def tail(n):
        # scale, wp, w sums, output DMAs
        st = state.pop(n)
        t, po, qq, tm, rdet = st["t"], st["po"], st["qq"], st["tm"], st["rdet"]
        for k in range(9):
            eng[plan["scale"]].tensor_mul(
                _V(po, k * c, [(1, c)]),
                _V(po, k * c, [(1, c)]),
                rdet[:],
            )
        for r in range(3):
            eng[plan[f"wp{r}"]].tensor_mul(
                _V(qq, 3 * r * c, [(1, 3 * c)]),
                _V(po, 3 * r * c, [(1, 3 * c)]),
                _V(t, 9 * c, [(1, 3 * c)]),
            )
        # O block can ship while the w tail computes
        nc.sync.dma_start(
            out=_V(out_t[n], 0, [(1, 9 * c)]), in_=_V(po, 0, [(1, 9 * c)])
        )
        # w_r = wp[3r] + wp[3r+1] + wp[3r+2] -> po planes 9..11
        # (s scratch reuses tm, dead after the det sums)
        eng[plan["s"]].tensor_add(
            _V(tm, 0, [(c, 3), (1, c)]),
            _V(qq, 0, [(3 * c, 3), (1, c)]),
            _V(qq, c, [(3 * c, 3), (1, c)]),
        )
        eng[plan["w"]].tensor_add(
            _V(po, 9 * c, [(c, 3), (1, c)]),
            _V(tm, 0, [(c, 3), (1, c)]),
            _V(qq, 2 * c, [(3 * c, 3), (1, c)]),
        )
        nc.sync.dma_start(
            out=_V(out_t[n], 9 * c, [(1, 3 * c)]),
            in_=_V(po, 9 * c, [(1, 3 * c)]),
        )

    with TileContext(nc) as tc:
        with (
            tc.tile_pool(name="tio", bufs=3) as tio,
            tc.tile_pool(name="io", bufs=2) as io,
            tc.tile_pool(name="tmp", bufs=2) as tmp,
        ):
            eng = {"v": nc.vector, "g": nc.gpsimd}
            # software pipeline: V runs chunk n+1's products while ACT does
            # chunk n's rdet9 fan-out, so V never stalls on the copy chain.
            # prefetch(n+1) is emitted after tail(n-1) so every prior-chunk
            # consumer of its recycled buffers is already known to Tile.
            prefetch(0, tio)
            for n in range(nch):
                head(n, io, tmp)
                if n >= 1:
                    tail(n - 1)
                mid(n, tmp)
                if n + 1 < nch:
                    prefetch(n + 1, tio)
            tail(nch - 1)

    return nc


_CACHE = {}


def _get_nc():
    if "nc" not in _CACHE:
        nc = build_nc()
        nc.finalize()
        _CACHE["nc"] = nc
    return _CACHE["nc"]


def _shard_inputs(trf):
    """(B,3,4) -> per-core (nch*128, 12*C) planar slabs."""
    x = np.ascontiguousarray(np.asarray(trf, dtype=np.float32)).reshape(
        NCORES, NCH, P, C, 12
    )
    # permute matrix positions into plane slots, planes outer, matrices inner
    xp = x[:, :, :, :, POS].transpose(0, 1, 2, 4, 3)  # (8, nch, 128, 12, C)
    xp = np.ascontiguousarray(xp).reshape(NCORES, NCH * P, 12 * C)
    return xp


def _unshard_output(outs):
    """per-core (nch*128, 12*C) planar -> (B, 3, 4)."""
    o = outs.reshape(NCORES, NCH, P, 12, C).transpose(0, 1, 2, 4, 3)
    full = np.empty((NCORES, NCH, P, C, 12), dtype=np.float32)
    full[..., OPOS] = o
    return full.reshape(B, 3, 4)


def run(trf, trace=False, **spmd_kwargs):
    """Shard, run on 8 cores, gather. Returns (output, BassKernelResults)."""
    from concourse.bass_utils import run_bass_kernel_spmd

    xp = _shard_inputs(trf)
    in_maps = [{"trf": xp[i]} for i in range(NCORES)]
    nc = _get_nc()
    res = run_bass_kernel_spmd(
        nc, in_maps, list(range(NCORES)), trace=trace, **spmd_kwargs
    )
    outs = np.stack([np.asarray(res.results[i]["out"]) for i in range(NCORES)])
    return _unshard_output(outs).astype(np.float32), res


def kernel(trf):
    return run(trf)[0]


# revision 25
# speedup vs baseline: 1.1893x; 1.0099x over previous
"""Trainium2 Bass kernel: batched inverse of homogeneous affine transforms.

Problem: trf (B, 3, 4) fp32 "shift" affines. Padded M = [[I3 + dA, t], [0, 1]].
Output = top 3 rows of M^-1 = [A^-1 | -A^-1 t] where A = I3 + dA.

Closed form via the column-cross-product adjugate:
    Z[3r+j]  = P[3r+j] - Q[3r+j]   (cross(a_{r+1}, a_{r+2}) components)
    det      = a1 . Z[0:3] ; O = Z * (1/det) ; w_r = sum_j O[r][j] * (-t_j)

Layout: PLANAR per partition. Host pre-permutes each core's (BL, 12) slab to
(nch, 128, 12, C): partition p holds 12 contiguous planes of C consecutive
matrices. Every engine op then runs on dense step-1 inner runs (measured ~2x
faster on DVE than the stride-12 interleaved layout) while each DMA still
moves one contiguous 24KB run per partition.

Plane permutation POS (slot -> matrix position) was chosen by combinatorial
search so the 9 Q-products batch as 3 ops, P row 0 batches, and tm batches
(arithmetic-progression plane strides).

Engine split: ALL 2-input tensor ops run on the DVE. GPSIMD shares its SBUF
port with the DVE (the POOL slot) — measured combined throughput with both
engines active equals DVE-alone, so offloading to GPSIMD is a net loss. ACT
(its own port) runs the 1-input work (diag+1, t negation) in parallel. The
det->recip->scale chain stays entirely on V (a cross-engine replication copy
costs ~1.2-1.8us in semaphore latency per chunk); emission is software-
pipelined so V runs chunk n+1's products while ACT works and DMAs stream.
"""

import numpy as np

B = 4_194_304
NCORES = 8
BL = B // NCORES  # 524288 matrices per core
P = 128
C = 512           # matrices per partition per chunk
NCH = BL // (P * C)  # 8 chunks

# slot -> input position (position = 4*r + col, row-major (3,4))
POS = [5, 0, 4, 8, 9, 10, 2, 6, 1, 3, 7, 11]
# output plane k -> output position: planes 0..8 = O[r][j] at 4r+j, 9..11 = w_r
OPOS = [0, 1, 2, 4, 5, 6, 8, 9, 10, 3, 7, 11]

# P products (out plane 3r+j in po block), as (out, in0_slot, in1_slot):
# row 0 batched: out {0,1,2}, in0 [0,4,8] (step 4), in1 [5,6,7] (step 1)
# rows 1,2 as pairs (2-term progressions are always affine):
# (out_base, out_step, in0_base, in0_step, in1_base, in1_step)
P_PAIRS = [
    (3, 1, 7, -2, 3, -2),   # (3,7,3), (4,5,1)
    (5, 1, 6, -4, 2, 2),    # (5,6,2), (6,2,4)
    (7, 1, 3, -2, 8, -8),   # (7,3,8), (8,1,0)
]
# Q products batched by j: (out_base, out_step, in0_base, in0_step, in1_base, in1_step)
Q_BATCH = [
    (0, 3, 7, -2, 4, -2),   # j=0: out {0,3,6}, in0 [7,5,3], in1 [4,2,0]
    (1, 3, 5, -2, 8, -2),   # j=1: out {1,4,7}, in0 [5,3,1], in1 [8,6,4]
    (2, 3, 0, 1, 6, 1),     # j=2: out {2,5,8}, in0 [0,1,2], in1 [6,7,8]
]

# engine plan: op -> "v" (DVE) / "g" (GPSIMD). GPSIMD's SBUF port is shared
# with the DVE (POOL slot): measured combined V+G throughput during overlap
# equals V-alone, so all 2-input work stays on V; ACT (own port) runs 1-input.
DEFAULT_PLAN = {
    "p0": "v",                               # batched P row 0
    **{f"pp{i}": "v" for i in range(3)},     # P pairs (rows 1,2)
    **{f"q{j}": "v" for j in range(3)},      # batched Q
    "z": "v",
    "tm": "v",
    "det1": "v",
    "det2": "v",
    "s": "v",
    "w": "v",
    "scale": "v",
    **{f"wp{r}": "v" for r in range(3)},
}


def _V(base_ap, off, dims):
    """Strided view of a tile: dims = [(step, count), ...] free dims, last
    dim innermost. Offset in elements."""
    import concourse.bass as bass

    return bass.AP(
        base_ap.tensor,
        base_ap.offset + off,
        [list(base_ap.ap[0])] + [[int(s), int(n)] for s, n in dims],
    )


def build_nc(bl=BL, c=C, plan=None):
    import concourse.bass as bass
    import concourse.bacc as bacc
    import concourse.mybir as mybir
    from concourse.tile import TileContext

    plan = dict(DEFAULT_PLAN, **(plan or {}))
    f32 = mybir.dt.float32
    nch = bl // (P * c)
    assert bl == nch * P * c

    nc = bacc.Bacc()
    # DRAM layout (host-permuted): (nch*128, 12*C) — row = (chunk, partition),
    # 12 planar planes of C floats contiguous per row.
    trf = nc.declare_dram_parameter("trf", [nch * P, 12 * c], f32, isOutput=False)
    out = nc.declare_dram_parameter("out", [nch * P, 12 * c], f32, isOutput=True)
    trf_t = trf.ap().rearrange("(n p) f -> n p f", p=P)
    out_t = out.ap().rearrange("(n p) f -> n p f", p=P)

    eng = None
    state = {}

    def prefetch(n, tio):
        # DMA-in + diag (ACT) — issued one stage early so the load and the
        # diag pass are done before the products need them
        t = tio.tile([P, 12 * c], f32, tag="t")
        if n == 0:
            # chunk 0 gates kernel startup: land the 4 planes the first
            # product op (pair pp0: slots {7,5}x{3,1}) needs first — odd
            # planes {1,3,5,7} are one stride-2C pattern — then the evens,
            # then the t-planes, with the diag passes as their planes arrive
            nc.sync.dma_start(
                out=_V(t, c, [(2 * c, 4), (1, c)]),
                in_=_V(trf_t[n], c, [(2 * c, 4), (1, c)]),
            )
            d15 = _V(t, c, [(4 * c, 2), (1, c)])
            nc.scalar.add(d15, d15, 1.0)
            nc.sync.dma_start(
                out=_V(t, 0, [(2 * c, 5), (1, c)]),
                in_=_V(trf_t[n], 0, [(2 * c, 5), (1, c)]),
            )
            d0 = _V(t, 0, [(1, c)])
            nc.scalar.add(d0, d0, 1.0)
            nc.sync.dma_start(
                out=_V(t, 9 * c, [(1, 3 * c)]),
                in_=_V(trf_t[n], 9 * c, [(1, 3 * c)]),
            )
        else:
            # split load: products only need planes 0-8, so they can start
            # before the t-planes (9-11) land
            nc.sync.dma_start(
                out=_V(t, 0, [(1, 9 * c)]), in_=_V(trf_t[n], 0, [(1, 9 * c)])
            )
            nc.sync.dma_start(
                out=_V(t, 9 * c, [(1, 3 * c)]),
                in_=_V(trf_t[n], 9 * c, [(1, 3 * c)]),
            )
            d01 = _V(t, 0, [(1, 2 * c)])
            nc.scalar.add(d01, d01, 1.0)
            d5 = _V(t, 5 * c, [(1, c)])
            nc.scalar.add(d5, d5, 1.0)
        state[n] = {"t": t}

    def head(n, io, tmp):
        # all 18 products (V); pairs first — pp0 touches only 4 planes, so
        # chunk 0's first compute starts as early as possible
        st = state[n]
        t = st["t"]
        po = io.tile([P, 12 * c], f32, tag="po")  # P/Z/O planes 0..8, w 9..11
        qq = tmp.tile([P, 9 * c], f32, tag="qq")  # Q then wp
        st["po"], st["qq"] = po, qq
        for i, (ob, os_, a0, s0, b0, s1) in enumerate(P_PAIRS):
            eng[plan[f"pp{i}"]].tensor_mul(
                _V(po, ob * c, [(os_ * c, 2), (1, c)]),
                _V(t, a0 * c, [(s0 * c, 2), (1, c)]),
                _V(t, b0 * c, [(s1 * c, 2), (1, c)]),
            )
        eng[plan["p0"]].tensor_mul(
            _V(po, 0, [(c, 3), (1, c)]),
            _V(t, 0, [(4 * c, 3), (1, c)]),
            _V(t, 5 * c, [(c, 3), (1, c)]),
        )
        for j, (ob, os_, a0, s0, b0, s1) in enumerate(Q_BATCH):
            eng[plan[f"q{j}"]].tensor_mul(
                _V(qq, ob * c, [(os_ * c, 3), (1, c)]),
                _V(t, a0 * c, [(s0 * c, 3), (1, c)]),
                _V(t, b0 * c, [(s1 * c, 3), (1, c)]),
            )

    def mid(n, tmp):
        # Z, det chain, recip (V); rdet9 fan-out + tneg (ACT)
        st = state[n]
        t, po, qq = st["t"], st["po"], st["qq"]
        pf = _V(po, 0, [(1, 9 * c)])
        eng[plan["z"]].tensor_sub(pf, pf, _V(qq, 0, [(1, 9 * c)]))

        tm = tmp.tile([P, 3 * c], f32, tag="tm")
        st["tm"] = tm
        eng[plan["tm"]].tensor_mul(
            _V(tm, 0, [(c, 3), (1, c)]),
            _V(t, c, [(c, 3), (1, c)]),
            _V(po, 0, [(c, 3), (1, c)]),
        )
        det = tmp.tile([P, c], f32, tag="det")
        eng[plan["det1"]].tensor_add(
            det[:], _V(tm, 0, [(1, c)]), _V(tm, c, [(1, c)])
        )
        eng[plan["det2"]].tensor_add(det[:], det[:], _V(tm, 2 * c, [(1, c)]))

        # rdet = 1/det (single custom DVE op, ~4e-6 rel err; det~1 so no
        # edge cases). No replication: the scale stage reads plane 0 directly
        # as nine single-plane ops, keeping the whole det->scale chain on V
        # (a cross-engine copy costs more in semaphore latency than the
        # extra op overheads).
        rdet = tmp.tile([P, c], f32, tag="rdet")
        st["rdet"] = rdet
        nc.vector.reciprocal_approx_fast(rdet[:], det[:])

        # tneg: t planes 9..11 *= -1 (in place, ACT)
        tp = _V(t, 9 * c, [(1, 3 * c)])
        nc.scalar.mul(tp, tp, -1.0)

    def tail(n):
        # scale, wp, w sums, output DMAs
        st = state.pop(n)
        t, po, qq, tm, rdet = st["t"], st["po"], st["qq"], st["tm"], st["rdet"]
        for k in range(9):
            eng[plan["scale"]].tensor_mul(
                _V(po, k * c, [(1, c)]),
                _V(po, k * c, [(1, c)]),
                rdet[:],
            )
        for r in range(3):
            eng[plan[f"wp{r}"]].tensor_mul(
                _V(qq, 3 * r * c, [(1, 3 * c)]),
                _V(po, 3 * r * c, [(1, 3 * c)]),
                _V(t, 9 * c, [(1, 3 * c)]),
            )
        # O block can ship while the w tail computes
        nc.sync.dma_start(
            out=_V(out_t[n], 0, [(1, 9 * c)]), in_=_V(po, 0, [(1, 9 * c)])
        )
        # w_r = wp[3r] + wp[3r+1] + wp[3r+2] -> po planes 9..11
        # (s scratch reuses tm, dead after the det sums)
        eng[plan["s"]].tensor_add(
            _V(tm, 0, [(c, 3), (1, c)]),
            _V(qq, 0, [(3 * c, 3), (1, c)]),
            _V(qq, c, [(3 * c, 3), (1, c)]),
        )
        eng[plan["w"]].tensor_add(
            _V(po, 9 * c, [(c, 3), (1, c)]),
            _V(tm, 0, [(c, 3), (1, c)]),
            _V(qq, 2 * c, [(3 * c, 3), (1, c)]),
        )
        nc.sync.dma_start(
            out=_V(out_t[n], 9 * c, [(1, 3 * c)]),
            in_=_V(po, 9 * c, [(1, 3 * c)]),
        )

    with TileContext(nc) as tc:
        with (
            tc.tile_pool(name="tio", bufs=3) as tio,
            tc.tile_pool(name="io", bufs=2) as io,
            tc.tile_pool(name="tmp", bufs=2) as tmp,
        ):
            eng = {"v": nc.vector, "g": nc.gpsimd}
            # software pipeline: V runs chunk n+1's products while ACT does
            # chunk n's rdet9 fan-out, so V never stalls on the copy chain.
            # prefetch(n+1) is emitted after tail(n-1) so every prior-chunk
            # consumer of its recycled buffers is already known to Tile.
            prefetch(0, tio)
            for n in range(nch):
                head(n, io, tmp)
                if n >= 1:
                    tail(n - 1)
                mid(n, tmp)
                if n + 1 < nch:
                    prefetch(n + 1, tio)
            tail(nch - 1)

    return nc


_CACHE = {}


def _get_nc():
    if "nc" not in _CACHE:
        nc = build_nc()
        nc.finalize()
        _CACHE["nc"] = nc
    return _CACHE["nc"]


def _shard_inputs(trf):
    """(B,3,4) -> per-core (nch*128, 12*C) planar slabs."""
    x = np.ascontiguousarray(np.asarray(trf, dtype=np.float32)).reshape(
        NCORES, NCH, P, C, 12
    )
    # permute matrix positions into plane slots, planes outer, matrices inner
    xp = x[:, :, :, :, POS].transpose(0, 1, 2, 4, 3)  # (8, nch, 128, 12, C)
    xp = np.ascontiguousarray(xp).reshape(NCORES, NCH * P, 12 * C)
    return xp


def _unshard_output(outs):
    """per-core (nch*128, 12*C) planar -> (B, 3, 4)."""
    o = outs.reshape(NCORES, NCH, P, 12, C).transpose(0, 1, 2, 4, 3)
    full = np.empty((NCORES, NCH, P, C, 12), dtype=np.float32)
    full[..., OPOS] = o
    return full.reshape(B, 3, 4)


def run(trf, trace=False, **spmd_kwargs):
    """Shard, run on 8 cores, gather. Returns (output, BassKernelResults)."""
    from concourse.bass_utils import run_bass_kernel_spmd

    xp = _shard_inputs(trf)
    in_maps = [{"trf": xp[i]} for i in range(NCORES)]
    nc = _get_nc()
    res = run_bass_kernel_spmd(
        nc, in_maps, list(range(NCORES)), trace=trace, **spmd_kwargs
    )
    outs = np.stack([np.asarray(res.results[i]["out"]) for i in range(NCORES)])
    return _unshard_output(outs).astype(np.float32), res


def kernel(trf):
    return run(trf)[0]


# revision 26
# speedup vs baseline: 1.1911x; 1.0015x over previous
"""Trainium2 Bass kernel: batched inverse of homogeneous affine transforms.

Problem: trf (B, 3, 4) fp32 "shift" affines. Padded M = [[I3 + dA, t], [0, 1]].
Output = top 3 rows of M^-1 = [A^-1 | -A^-1 t] where A = I3 + dA.

Closed form via the column-cross-product adjugate:
    Z[3r+j]  = P[3r+j] - Q[3r+j]   (cross(a_{r+1}, a_{r+2}) components)
    det      = a1 . Z[0:3] ; O = Z * (1/det) ; w_r = sum_j O[r][j] * (-t_j)

Layout: PLANAR per partition. Host pre-permutes each core's (BL, 12) slab to
(nch, 128, 12, C): partition p holds 12 contiguous planes of C consecutive
matrices. Every engine op then runs on dense step-1 inner runs (measured ~2x
faster on DVE than the stride-12 interleaved layout) while each DMA still
moves one contiguous 24KB run per partition.

Plane permutation POS (slot -> matrix position) was chosen by combinatorial
search so the 9 Q-products batch as 3 ops, P row 0 batches, and tm batches
(arithmetic-progression plane strides).

Engine split: ALL 2-input tensor ops run on the DVE. GPSIMD shares its SBUF
port with the DVE (the POOL slot) — measured combined throughput with both
engines active equals DVE-alone, so offloading to GPSIMD is a net loss. ACT
(its own port) runs the 1-input work (diag+1, t negation) in parallel. The
det->recip->scale chain stays entirely on V (a cross-engine replication copy
costs ~1.2-1.8us in semaphore latency per chunk); emission is software-
pipelined so V runs chunk n+1's products while ACT works and DMAs stream.
"""

import numpy as np

B = 4_194_304
NCORES = 8
BL = B // NCORES  # 524288 matrices per core
P = 128
C = 512           # matrices per partition per chunk
NCH = BL // (P * C)  # 8 chunks

# slot -> input position (position = 4*r + col, row-major (3,4))
POS = [5, 0, 4, 8, 9, 10, 2, 6, 1, 3, 7, 11]
# output plane k -> output position: planes 0..8 = O[r][j] at 4r+j, 9..11 = w_r
OPOS = [0, 1, 2, 4, 5, 6, 8, 9, 10, 3, 7, 11]

# P products (out plane 3r+j in po block), as (out, in0_slot, in1_slot):
# row 0 batched: out {0,1,2}, in0 [0,4,8] (step 4), in1 [5,6,7] (step 1)
# rows 1,2 as pairs (2-term progressions are always affine):
# (out_base, out_step, in0_base, in0_step, in1_base, in1_step)
P_PAIRS = [
    (3, 1, 7, -2, 3, -2),   # (3,7,3), (4,5,1)
    (5, 1, 6, -4, 2, 2),    # (5,6,2), (6,2,4)
    (7, 1, 3, -2, 8, -8),   # (7,3,8), (8,1,0)
]
# Q products batched by j: (out_base, out_step, in0_base, in0_step, in1_base, in1_step)
Q_BATCH = [
    (0, 3, 7, -2, 4, -2),   # j=0: out {0,3,6}, in0 [7,5,3], in1 [4,2,0]
    (1, 3, 5, -2, 8, -2),   # j=1: out {1,4,7}, in0 [5,3,1], in1 [8,6,4]
    (2, 3, 0, 1, 6, 1),     # j=2: out {2,5,8}, in0 [0,1,2], in1 [6,7,8]
]

# engine plan: op -> "v" (DVE) / "g" (GPSIMD). GPSIMD's SBUF port is shared
# with the DVE (POOL slot): measured combined V+G throughput during overlap
# equals V-alone, so all 2-input work stays on V; ACT (own port) runs 1-input.
DEFAULT_PLAN = {
    "p0": "v",                               # batched P row 0
    **{f"pp{i}": "v" for i in range(3)},     # P pairs (rows 1,2)
    **{f"q{j}": "v" for j in range(3)},      # batched Q
    "z": "v",
    "tm": "v",
    "det1": "v",
    "det2": "v",
    "s": "v",
    "w": "v",
    "scale": "v",
    **{f"wp{r}": "v" for r in range(3)},
}


def _V(base_ap, off, dims):
    """Strided view of a tile: dims = [(step, count), ...] free dims, last
    dim innermost. Offset in elements."""
    import concourse.bass as bass

    return bass.AP(
        base_ap.tensor,
        base_ap.offset + off,
        [list(base_ap.ap[0])] + [[int(s), int(n)] for s, n in dims],
    )


def build_nc(bl=BL, c=C, plan=None):
    import concourse.bass as bass
    import concourse.bacc as bacc
    import concourse.mybir as mybir
    from concourse.tile import TileContext

    plan = dict(DEFAULT_PLAN, **(plan or {}))
    f32 = mybir.dt.float32
    nch = bl // (P * c)
    assert bl == nch * P * c

    nc = bacc.Bacc()
    # DRAM layout (host-permuted): (nch*128, 12*C) — row = (chunk, partition),
    # 12 planar planes of C floats contiguous per row.
    trf = nc.declare_dram_parameter("trf", [nch * P, 12 * c], f32, isOutput=False)
    out = nc.declare_dram_parameter("out", [nch * P, 12 * c], f32, isOutput=True)
    trf_t = trf.ap().rearrange("(n p) f -> n p f", p=P)
    out_t = out.ap().rearrange("(n p) f -> n p f", p=P)

    eng = None
    state = {}

    def prefetch(n, tio):
        # DMA-in + diag (ACT) — issued one stage early so the load and the
        # diag pass are done before the products need them
        t = tio.tile([P, 12 * c], f32, tag="t")
        if n == 0:
            # chunk 0 gates kernel startup: land the 4 planes the first
            # product op (pair pp0: slots {7,5}x{3,1}) needs first — odd
            # planes {1,3,5,7} are one stride-2C pattern — then the evens,
            # then the t-planes, with the diag passes as their planes arrive
            nc.sync.dma_start(
                out=_V(t, c, [(2 * c, 4), (1, c)]),
                in_=_V(trf_t[n], c, [(2 * c, 4), (1, c)]),
            )
            d15 = _V(t, c, [(4 * c, 2), (1, c)])
            nc.scalar.add(d15, d15, 1.0)
            nc.sync.dma_start(
                out=_V(t, 0, [(2 * c, 5), (1, c)]),
                in_=_V(trf_t[n], 0, [(2 * c, 5), (1, c)]),
            )
            d0 = _V(t, 0, [(1, c)])
            nc.scalar.add(d0, d0, 1.0)
            nc.sync.dma_start(
                out=_V(t, 9 * c, [(1, 3 * c)]),
                in_=_V(trf_t[n], 9 * c, [(1, 3 * c)]),
            )
        else:
            # split load: products only need planes 0-8, so they can start
            # before the t-planes (9-11) land
            nc.sync.dma_start(
                out=_V(t, 0, [(1, 9 * c)]), in_=_V(trf_t[n], 0, [(1, 9 * c)])
            )
            nc.sync.dma_start(
                out=_V(t, 9 * c, [(1, 3 * c)]),
                in_=_V(trf_t[n], 9 * c, [(1, 3 * c)]),
            )
            d01 = _V(t, 0, [(1, 2 * c)])
            nc.scalar.add(d01, d01, 1.0)
            d5 = _V(t, 5 * c, [(1, c)])
            nc.scalar.add(d5, d5, 1.0)
        state[n] = {"t": t}

    def head(n, io, tmp):
        # all 18 products (V); pairs first — pp0 touches only 4 planes, so
        # chunk 0's first compute starts as early as possible
        st = state[n]
        t = st["t"]
        po = io.tile([P, 12 * c], f32, tag="po")  # P/Z/O planes 0..8, w 9..11
        qq = tmp.tile([P, 9 * c], f32, tag="qq")  # Q then wp
        st["po"], st["qq"] = po, qq
        for i, (ob, os_, a0, s0, b0, s1) in enumerate(P_PAIRS):
            eng[plan[f"pp{i}"]].tensor_mul(
                _V(po, ob * c, [(os_ * c, 2), (1, c)]),
                _V(t, a0 * c, [(s0 * c, 2), (1, c)]),
                _V(t, b0 * c, [(s1 * c, 2), (1, c)]),
            )
        eng[plan["p0"]].tensor_mul(
            _V(po, 0, [(c, 3), (1, c)]),
            _V(t, 0, [(4 * c, 3), (1, c)]),
            _V(t, 5 * c, [(c, 3), (1, c)]),
        )
        for j, (ob, os_, a0, s0, b0, s1) in enumerate(Q_BATCH):
            eng[plan[f"q{j}"]].tensor_mul(
                _V(qq, ob * c, [(os_ * c, 3), (1, c)]),
                _V(t, a0 * c, [(s0 * c, 3), (1, c)]),
                _V(t, b0 * c, [(s1 * c, 3), (1, c)]),
            )

    def mid(n, tmp):
        # Z, det chain, recip (V); rdet9 fan-out + tneg (ACT)
        st = state[n]
        t, po, qq = st["t"], st["po"], st["qq"]
        pf = _V(po, 0, [(1, 9 * c)])
        eng[plan["z"]].tensor_sub(pf, pf, _V(qq, 0, [(1, 9 * c)]))

        tm = tmp.tile([P, 3 * c], f32, tag="tm")
        st["tm"] = tm
        eng[plan["tm"]].tensor_mul(
            _V(tm, 0, [(c, 3), (1, c)]),
            _V(t, c, [(c, 3), (1, c)]),
            _V(po, 0, [(c, 3), (1, c)]),
        )
        det = tmp.tile([P, c], f32, tag="det")
        eng[plan["det1"]].tensor_add(
            det[:], _V(tm, 0, [(1, c)]), _V(tm, c, [(1, c)])
        )
        eng[plan["det2"]].tensor_add(det[:], det[:], _V(tm, 2 * c, [(1, c)]))

        # rdet = 1/det (single custom DVE op, ~4e-6 rel err; det~1 so no
        # edge cases). No replication: the scale stage reads plane 0 directly
        # as nine single-plane ops, keeping the whole det->scale chain on V
        # (a cross-engine copy costs more in semaphore latency than the
        # extra op overheads).
        rdet = tmp.tile([P, c], f32, tag="rdet")
        st["rdet"] = rdet
        nc.vector.reciprocal_approx_fast(rdet[:], det[:])

        # tneg: t planes 9..11 *= -1 (in place, ACT)
        tp = _V(t, 9 * c, [(1, 3 * c)])
        nc.scalar.mul(tp, tp, -1.0)

    def tail(n):
        # scale, wp, w sums, output DMAs
        st = state.pop(n)
        t, po, qq, tm, rdet = st["t"], st["po"], st["qq"], st["tm"], st["rdet"]
        for k in range(9):
            eng[plan["scale"]].tensor_mul(
                _V(po, k * c, [(1, c)]),
                _V(po, k * c, [(1, c)]),
                rdet[:],
            )
        for r in range(3):
            eng[plan[f"wp{r}"]].tensor_mul(
                _V(qq, 3 * r * c, [(1, 3 * c)]),
                _V(po, 3 * r * c, [(1, 3 * c)]),
                _V(t, 9 * c, [(1, 3 * c)]),
            )
        # O block can ship while the w tail computes
        nc.sync.dma_start(
            out=_V(out_t[n], 0, [(1, 9 * c)]), in_=_V(po, 0, [(1, 9 * c)])
        )
        # w_r = wp[3r] + wp[3r+1] + wp[3r+2] -> po planes 9..11
        # (s scratch reuses tm, dead after the det sums)
        eng[plan["s"]].tensor_add(
            _V(tm, 0, [(c, 3), (1, c)]),
            _V(qq, 0, [(3 * c, 3), (1, c)]),
            _V(qq, c, [(3 * c, 3), (1, c)]),
        )
        eng[plan["w"]].tensor_add(
            _V(po, 9 * c, [(c, 3), (1, c)]),
            _V(tm, 0, [(c, 3), (1, c)]),
            _V(qq, 2 * c, [(3 * c, 3), (1, c)]),
        )
        nc.sync.dma_start(
            out=_V(out_t[n], 9 * c, [(1, 3 * c)]),
            in_=_V(po, 9 * c, [(1, 3 * c)]),
        )

    with TileContext(nc) as tc:
        with (
            tc.tile_pool(name="tio", bufs=3) as tio,
            tc.tile_pool(name="io", bufs=2) as io,
            tc.tile_pool(name="tmp", bufs=2) as tmp,
        ):
            eng = {"v": nc.vector, "g": nc.gpsimd}
            # software pipeline: head(n+1)'s products sit between mid(n) and
            # tail(n) in V's queue, covering ACT latency and DMA landings.
            # prefetch(n+1) is emitted after tail(n-1) so every prior-chunk
            # consumer of its recycled buffers is already known to Tile.
            prefetch(0, tio)
            for n in range(nch):
                head(n, io, tmp)
                if n >= 1:
                    tail(n - 1)
                mid(n, tmp)
                if n + 1 < nch:
                    prefetch(n + 1, tio)
            tail(nch - 1)

    return nc


_CACHE = {}


def _get_nc():
    if "nc" not in _CACHE:
        nc = build_nc()
        nc.finalize()
        _CACHE["nc"] = nc
    return _CACHE["nc"]


def _shard_inputs(trf):
    """(B,3,4) -> per-core (nch*128, 12*C) planar slabs."""
    x = np.ascontiguousarray(np.asarray(trf, dtype=np.float32)).reshape(
        NCORES, NCH, P, C, 12
    )
    # permute matrix positions into plane slots, planes outer, matrices inner
    xp = x[:, :, :, :, POS].transpose(0, 1, 2, 4, 3)  # (8, nch, 128, 12, C)
    xp = np.ascontiguousarray(xp).reshape(NCORES, NCH * P, 12 * C)
    return xp


def _unshard_output(outs):
    """per-core (nch*128, 12*C) planar -> (B, 3, 4)."""
    o = outs.reshape(NCORES, NCH, P, 12, C).transpose(0, 1, 2, 4, 3)
    full = np.empty((NCORES, NCH, P, C, 12), dtype=np.float32)
    full[..., OPOS] = o
    return full.reshape(B, 3, 4)


def run(trf, trace=False, **spmd_kwargs):
    """Shard, run on 8 cores, gather. Returns (output, BassKernelResults)."""
    from concourse.bass_utils import run_bass_kernel_spmd

    xp = _shard_inputs(trf)
    in_maps = [{"trf": xp[i]} for i in range(NCORES)]
    nc = _get_nc()
    res = run_bass_kernel_spmd(
        nc, in_maps, list(range(NCORES)), trace=trace, **spmd_kwargs
    )
    outs = np.stack([np.asarray(res.results[i]["out"]) for i in range(NCORES)])
    return _unshard_output(outs).astype(np.float32), res


def kernel(trf):
    return run(trf)[0]
